# revision 49
# baseline (speedup 1.0000x reference)
"""Mip-NeRF ray marcher (MipRayMarcher2) as a Trainium2 Bass/Tile kernel.

Problem: B=4, R=16384, S=96 samples/ray. Per ray (along samples):
    deltas    = diff(depths)
    mids      = midpoints of colors / densities / depths
    dd        = softplus(densities_mid) * deltas
    alpha     = 1 - exp(-dd)
    T         = exclusive cumprod of (1 - alpha)  (transmittance)
    weights   = alpha * T
    rgb       = sum(weights * colors_mid);  depth = clip(sum(weights*depths_mid))

Device formulation (mathematically identical, fp32-equivalent):
    csum      = per-group inclusive cumsum of dd, done with ONE
                tensor_tensor_scan per tile: the 98-wide per-group segment is
                [sep, sep, dd0..dd94, dup] and the scan recurrence
                state = data0*state + data1 resets to 0 at the two separator
                columns (data0=0 there) and replays the last value at `dup`.
    Spad      = exp(-csum) = [1, 1, S1..S95, S95] per group (separators
                become the leading ones for free)
    weights[i]= Spad[i+1] - Spad[i+2]
    v[s]      = w[s-1] + w[s] = Spad[s] - Spad[s+2]   (telescoped)
    rgb       = sum_s (0.5*colors[s]) * v[s]          (midpoint re-associated)
    depth     = sum_s (0.5*depths[s]) * v[s], clipped to global [min,max]

Sharding: fully data-parallel over rays; 65536 rays -> 8 cores x 8192 rays.

DMA layout notes: densities+depths are packed host-side into one
[rays, 2, 96] tensor so each DMA row is 768 B (>= the 512 B SDMA line-rate
threshold); weights are written to a [rays, 128] padded DRAM tensor (512 B
rows) and the host strips the pad.  Colors rows are already 1152 B.

Engine split: GPSIMD does the midpoint add, delta sub and weight sub; ACT
does exp/ln (softplus = ln(1+exp(x)) so only the natural_log_exp_and_others
table set is ever needed -- get_activation_tables is patched below so the
greedy table chooser cannot pick the exp-only/ln-only sets, which would
otherwise thrash ~2.7us table loads every iteration); DVE does the dd mul,
the scan, v, the compositing muls, reduces and the clip.
"""

import numpy as np
from contextlib import ExitStack

import concourse.bass as bass
import concourse.hw_specs as hw_specs
import concourse.bacc as bacc
import concourse.tile as tile
from concourse import mybir
from concourse.bass_utils import run_bass_kernel_spmd

F32 = mybir.dt.float32
AF = mybir.ActivationFunctionType
OP = mybir.AluOpType

B, R, S = 4, 16384, 96
SM = S - 1                 # 95 intervals
N_CORES = 8
RAYS = B * R               # 65536
RPC = RAYS // N_CORES      # 8192 rays per core
P = 128                    # SBUF partitions
G = 4                      # ray groups (of 128 rays) per supertile
RAYS_PER_TILE = P * G      # 1024
NT = RPC // RAYS_PER_TILE  # 8 supertiles per core
SEG = S + 2                # per-group scan segment: [sep, sep, dd0..dd94, dup]
FSEG = G * SEG             # flat segment length = 784
WPAD = 128                 # padded weights row (512 B)

PROFILE = False            # set by test.py (kept for compatibility)
LAST_RESULTS = None

_PROGRAM_CACHE = {}


def _patch_act_tables():
    """Keep only natural_log_exp_and_others as a provider of Exp/Ln so the
    table-load chooser never alternates between the exp-only and ln-only
    sets.  Indices (act_func_set_id) are preserved: only membership of the
    other sets is trimmed."""
    if getattr(bacc, "_act_tables_patched", False):
        return
    orig = hw_specs.get_activation_tables

    def patched(arch):
        t = orig(arch)
        keep = "natural_log_exp_and_others"
        if keep in t:
            for name, funcs in t.items():
                if name != keep:
                    funcs.discard(AF.Exp)
                    funcs.discard(AF.Ln)
        return t

    bacc.get_activation_tables = patched
    bacc._act_tables_patched = True


def _build_program():
    _patch_act_tables()
    nc = bacc.Bacc("TRN2", target_bir_lowering=False, debug=False, enable_asserts=False)

    col_d = nc.declare_dram_parameter("colors", [RPC, S, 3], F32, isOutput=False)
    dd_d = nc.declare_dram_parameter("dendep", [RPC, 2, S], F32, isOutput=False)
    bnd_d = nc.declare_dram_parameter("bounds", [P, 2], F32, isOutput=False)
    w_d = nc.declare_dram_parameter("weights", [RPC, WPAD], F32, isOutput=True)
    rgb_d = nc.declare_dram_parameter("rgb", [RPC, 3], F32, isOutput=True)
    dpt_d = nc.declare_dram_parameter("depth", [RPC, 1], F32, isOutput=True)

    with tile.TileContext(nc) as tc, ExitStack() as ctx:
        consts = ctx.enter_context(tc.tile_pool(name="consts", bufs=1))
        loads = ctx.enter_context(tc.tile_pool(name="loads", bufs=4))
        work = ctx.enter_context(tc.tile_pool(name="work", bufs=4))
        outp = ctx.enter_context(tc.tile_pool(name="outp", bufs=4))

        # scan data0: 1 everywhere, 0 at the two leading separator columns of
        # each 98-wide group segment.  state = data0*state + data1 gives a
        # per-group cumsum that resets to 0 at each group boundary and
        # replays the final value at the trailing `dup` column.
        ones01 = consts.tile([P, G, SEG], F32)
        nc.vector.memset(ones01[:], 1.0)
        nc.vector.memset(ones01[:, :, 0:2], 0.0)

        # persistent scan data1 buffer; separator/dup columns zeroed once,
        # dd rewritten into [:, :, 2:97] each tile
        ddb = consts.tile([P, G, SEG], F32)
        nc.vector.memset(ddb[:], 0.0)
        ddb_flat = ddb[:].rearrange("p g s -> p (g s)")

        bnd = consts.tile([P, 2], F32)
        nc.sync.dma_start(out=bnd[:], in_=bnd_d.ap())

        for t in range(NT):
            r0 = t * RAYS_PER_TILE
            r1 = r0 + RAYS_PER_TILE

            col = loads.tile([P, G, S, 3], F32)
            nc.sync.dma_start(
                out=col[:],
                in_=col_d.ap()[r0:r1].rearrange("(g p) s c -> p g s c", p=P),
            )
            ddt = loads.tile([P, G, 2, S], F32)
            nc.sync.dma_start(
                out=ddt[:],
                in_=dd_d.ap()[r0:r1].rearrange("(g p) h s -> p g h s", p=P),
            )
            den3 = ddt[:, :, 0, :]
            dep3 = ddt[:, :, 1, :]

            # densities midpoint sum / depth deltas (GPSIMD)
            da = work.tile([P, G, SM], F32)
            nc.gpsimd.tensor_add(da[:], den3[:, :, 0:SM], den3[:, :, 1:S])
            dl = work.tile([P, G, SM], F32)
            nc.gpsimd.tensor_sub(dl[:], dep3[:, :, 1:S], dep3[:, :, 0:SM])

            # softplus(0.5*da) = ln(1 + exp(0.5*da))
            da_f = da[:].rearrange("p g s -> p (g s)")
            e1 = work.tile([P, G * SM], F32)
            nc.scalar.activation(e1[:], da_f, AF.Exp, bias=0.0, scale=0.5)
            # softplus in-place: sp = ln(e1 + 1) overwrites e1
            nc.scalar.activation(e1[:], e1[:], AF.Ln, bias=1.0, scale=1.0)

            # dd -> persistent segment buffer [sep, sep, dd0..94, dup]/group
            sp3 = e1[:].rearrange("p (g s) -> p g s", g=G)
            nc.gpsimd.tensor_mul(ddb[:, :, 2 : 2 + SM], sp3, dl[:])

            # one scan for all G groups: state = data0*state + data1
            csum = work.tile([P, FSEG], F32)
            nc.vector.tensor_tensor_scan(
                csum[:],
                ones01[:].rearrange("p g s -> p (g s)"),
                ddb_flat,
                0.0,
                OP.mult,
                OP.add,
            )

            # Spad per group = [1, 1, S1..S95, S95]
            spad = work.tile([P, FSEG], F32)
            nc.scalar.activation(spad[:], csum[:], AF.Exp, bias=0.0, scale=-1.0)
            spad3 = spad[:].rearrange("p (g s) -> p g s", g=G)

            # weights (GPSIMD) into a 128-wide padded tile -> 512 B DMA rows
            wtp = outp.tile([P, G, WPAD], F32)
            nc.gpsimd.tensor_sub(
                wtp[:, :, 0:SM], spad3[:, :, 1 : 1 + SM], spad3[:, :, 2 : 2 + SM]
            )
            nc.gpsimd.memset(wtp[:, :, SM:WPAD], 0.0)
            nc.scalar.dma_start(
                out=w_d.ap()[r0:r1].rearrange("(g p) s -> p g s", p=P),
                in_=wtp[:],
            )

            # v[s] = Spad[s] - Spad[s+2]  (= w[s-1] + w[s], telescoped)
            v = work.tile([P, G, S], F32)
            nc.vector.tensor_sub(v[:], spad3[:, :, 0:S], spad3[:, :, 2:SEG])

            # rgb: sum_s (0.5*colors) * v (per channel), one batched reduce
            # channel 0 on GPSIMD as a plain mul (no stt opcode on Pool);
            # its missing 0.5 is applied post-reduce on the idle ACT engine
            prt = work.tile([P, G, 3, S], F32)
            nc.gpsimd.tensor_mul(prt[:, :, 0, :], col[:, :, :, 0], v[:])
            for c in (1, 2):
                nc.vector.scalar_tensor_tensor(
                    prt[:, :, c, :], col[:, :, :, c], 0.5, v[:], OP.mult, OP.mult
                )
            rgbo = outp.tile([P, G, 3], F32)
            nc.vector.tensor_reduce(
                rgbo[:], prt[:], axis=mybir.AxisListType.X, op=OP.add
            )
            nc.vector.tensor_scalar_mul(rgbo[:, :, 0], rgbo[:, :, 0], 0.5)
            nc.scalar.dma_start(
                out=rgb_d.ap()[r0:r1].rearrange("(g p) c -> p g c", p=P),
                in_=rgbo[:],
            )

            # depth: sum_s (0.5*depths) * v into a contiguous tile (2x-mode
            # eligible reduce), product on GPSIMD
            pdt = work.tile([P, G, S], F32)
            nc.vector.scalar_tensor_tensor(
                pdt[:], dep3, 0.5, v[:], OP.mult, OP.mult
            )
            dsum = work.tile([P, G, 1], F32)
            nc.vector.tensor_reduce(
                dsum[:], pdt[:], axis=mybir.AxisListType.X, op=OP.add
            )
            dcl = outp.tile([P, G, 1], F32)
            nc.vector.tensor_scalar(
                dcl[:], dsum[:], bnd[:, 0:1], bnd[:, 1:2], OP.max, OP.min
            )
            nc.scalar.dma_start(
                out=dpt_d.ap()[r0:r1].rearrange("(g p) c -> p g c", p=P),
                in_=dcl[:],
            )

    import os as _os
    if not _os.environ.get("KERNEL_SKIP_COMPILE"):
        nc.compile()
    return nc


def kernel(colors, densities, depths, white_back=0):
    global LAST_RESULTS
    colors = np.ascontiguousarray(
        np.asarray(colors, dtype=np.float32).reshape(RAYS, S, 3)
    )
    densities = np.asarray(densities, dtype=np.float32).reshape(RAYS, S)
    depths = np.asarray(depths, dtype=np.float32).reshape(RAYS, S)

    dendep = np.empty((RAYS, 2, S), np.float32)
    dendep[:, 0, :] = densities
    dendep[:, 1, :] = depths

    # global clip bounds for composite_depth (depths are sorted along samples)
    lo = float(depths[:, 0].min())
    hi = float(depths[:, -1].max())
    bounds = np.empty((P, 2), np.float32)
    bounds[:, 0] = lo
    bounds[:, 1] = hi

    if "prog" not in _PROGRAM_CACHE:
        _PROGRAM_CACHE["prog"] = _build_program()
    nc = _PROGRAM_CACHE["prog"]

    in_maps = []
    for i in range(N_CORES):
        sl = slice(i * RPC, (i + 1) * RPC)
        in_maps.append(
            {
                "colors": colors[sl],
                "dendep": dendep[sl],
                "bounds": bounds,
            }
        )

    res = run_bass_kernel_spmd(nc, in_maps, list(range(N_CORES)))
    LAST_RESULTS = res

    rgb = np.concatenate([np.asarray(r["rgb"]) for r in res.results]).reshape(
        B, R, 3
    )
    depth = np.concatenate(
        [np.asarray(r["depth"]) for r in res.results]
    ).reshape(B, R, 1)
    weights = np.concatenate(
        [np.asarray(r["weights"])[:, 0:SM] for r in res.results]
    ).reshape(B, R, SM, 1)

    if int(white_back):
        rgb = rgb + 1.0 - weights.sum(axis=2)

    return (
        rgb.astype(np.float32),
        depth.astype(np.float32),
        weights.astype(np.float32),
    )
